# revision 1
# baseline (speedup 1.0000x reference)
"""CFConv (gnn message passing) Trainium2 kernel.

Math (per batch b):
    f1 = ssp(r @ W1 + b1)            ssp(x) = softplus(x) - log2
    f2 = ssp(f1 @ W2 + b2)
    out[i, d] = sum_j x[j, d] * f2[i, j, d]

Sharding: data-parallel over batch B=8 across the 8 cores (one batch each).

Per-core device pipeline (all in "transposed" orientation, features on
partitions, (i,j)-rows on the free dim):
  1. r rows are viewed as pairs [(i,j2), 128] in bf16, transposed on the
     HOST (numpy) to rpt [128, pairs], and loaded with plain wide DMA
     into SBUF tiles [128, chunk]; partitions 0:64 hold the rbf features
     of even j, 64:128 of odd j.  (The on-device xbar transpose ran at
     ~33 GB/s and rate-limited the whole pipeline; plain loads don't.)
  2. mm1: two K=64 row-tiled matmuls against a stacked [W1;W1] stationary
     (they run concurrently in different PE row groups).
  3. act1: softplus as Ln(Exp(z1 + b1) + 1) -- two ACT passes (no native
     softplus table on this stack; Exp and Ln share one ACT table set, and
     the table chooser is pinned to it to avoid per-op table reloads).
     Exp goes PSUM -> SBUF f32 per group; Ln runs once per 4-group chunk
     (FD=8192) to amortize the ~0.5 us per-op ACT bubble.
     The "- log2" shift is folded into layer 2's bias (b2' below).
  4. mm2: K=128 matmuls against W2.
  5. act2: same two-pass softplus with bias b2' = b2 - log2 * sum_d W2[d,:].
  6. Final contraction over j on DVE: prod = a2 * xT (xT broadcast over i),
     even+odd TT add, one more j-halving TT add (TT adds run ~2x the
     elem rate of TensorReduce), a segmented tensor_reduce over the
     remaining j, then a per-partition add of the "- log2 * sum_j x[j,d]"
     correction.  Output stays [d, i] on device; the host transposes back.

Emission is software-pipelined: layer-2 work of chunk c-1 interleaves with
layer-1 work of chunk c at group granularity, so the ACT engine (the
bottleneck at ~1 elem/lane/cycle x 4 passes, ~95% occupancy) never
head-of-line blocks on matmuls.  Ln1 is split [g0 | g1:] so the next
iteration's mm2 gets PE work during the Ln block (avoids cold-p-state
matmul chains at chunk boundaries); the last chunk + flush run their
Ln2/contraction per half-group so the post-ACT DVE overhang at the end
is one 1024-col piece.  Constants are packed into two host-side blobs
(DMA issue costs ~650 ns of serial queue time each), and r is transposed
on the host so the device does plain wide loads (the on-device xbar
transpose ran at ~33 GB/s and rate-limited the old pipeline).
TimelineSim: ~252 us per core (all 8 cores run in parallel, one batch
each).
"""

import numpy as np
import ml_dtypes

import concourse.bass as bass
import concourse.tile as tile
from concourse import bacc, mybir
from concourse.bass_utils import run_bass_kernel_spmd

LOG2 = float(np.log(2.0))

B, N, D, RBF = 8, 256, 128, 64
PAIRS = N * N // 2            # 32768 row-pairs per batch
CHUNK_PAIRS = 4096            # pairs per DMA-transpose chunk (1 MiB)
GROUP_PAIRS = 1024            # pairs per PSUM group (8 query nodes i)
SUB = 512                     # pairs per matmul (one PSUM bank)
I_PER_GROUP = GROUP_PAIRS // (N // 2)   # 8
H = CHUNK_PAIRS // GROUP_PAIRS          # groups per chunk tile (4)
N_CORES = 8

BF16 = mybir.dt.bfloat16
F32 = mybir.dt.float32


def _build_program(reps: int = 1):
    # Restrict the ACT-table chooser to the one set holding BOTH Exp and Ln;
    # otherwise it alternates between per-function sets and pays a ~2.7us
    # table load on every activation.
    import concourse.bacc as _bacc_mod
    from concourse.hw_specs import get_activation_tables as _gat
    _orig = _gat("gen3")
    _both = mybir.ActivationFunctionType.Exp, mybir.ActivationFunctionType.Ln
    _patched = {
        name: (funcs if name == "natural_log_exp_and_others"
               else type(funcs)(f for f in funcs if f not in _both))
        for name, funcs in _orig.items()
    }
    _bacc_mod.get_activation_tables = lambda arch: _patched

    nc = bacc.Bacc("TRN2", target_bir_lowering=False, debug=False,
                   num_devices=N_CORES)

    rpt = nc.dram_tensor("rpt", [2 * RBF, PAIRS], BF16, kind="ExternalInput").ap()
    # Constants packed into two blobs (one DMA each): DMA issue costs
    # ~650 ns of serial SP-queue time per op, so 7 separate const loads
    # would push the first r chunk out by ~4.5 us.
    # cb16 cols: [w1s | w2 | xte | xto], cf32 cols: [b1, b2p, corr].
    cb16 = nc.dram_tensor("cb16", [D, 4 * D], BF16, kind="ExternalInput").ap()
    cf32 = nc.dram_tensor("cf32", [D, 3], F32, kind="ExternalInput").ap()
    outT = nc.dram_tensor("outT", [D, N], F32, kind="ExternalOutput").ap()

    f_exp = mybir.ActivationFunctionType.Exp
    f_ln = mybir.ActivationFunctionType.Ln
    mult = mybir.AluOpType.mult
    add = mybir.AluOpType.add

    with tile.TileContext(nc) as tc:
        with (
            tc.tile_pool(name="const", bufs=1) as const,
            tc.tile_pool(name="rt", bufs=2) as rt_pool,
            tc.tile_pool(name="e1", bufs=1) as e1_pool,
            tc.tile_pool(name="e2", bufs=1) as e2_pool,
            tc.tile_pool(name="a1", bufs=2) as a1_pool,
            tc.tile_pool(name="a2", bufs=2) as a2_pool,
            tc.tile_pool(name="prod", bufs=2) as prod_pool,
            tc.tile_pool(name="acc", bufs=2) as acc_pool,
            tc.tile_pool(name="osb", bufs=1) as out_pool,
            tc.tile_pool(name="f1", bufs=1, space="PSUM") as f1_pool,
            tc.tile_pool(name="f2", bufs=1, space="PSUM") as f2_pool,
        ):
            cb16_t = const.tile([D, 4 * D], BF16, tag="cb16")
            cf32_t = const.tile([D, 3], F32, tag="cf32")
            w1s_t = cb16_t[:, 0 * D:1 * D]
            w2_t = cb16_t[:, 1 * D:2 * D]
            xte_t = cb16_t[:, 2 * D:2 * D + N // 2]
            xto_t = cb16_t[:, 3 * D:3 * D + N // 2]
            b1_t = cf32_t[:, 0:1]
            b2p_t = cf32_t[:, 1:2]
            corr_t = cf32_t[:, 2:3]

            out_sb = out_pool.tile([D, N], F32, tag="osb")

            # Warm activation fed by a DVE memzero: its only dep clears at
            # t~0.1us, so the ACT-table load (1.3us) runs during the input
            # DMAs instead of in front of the first real Exp.
            warm = acc_pool.tile([D, 1], F32, tag="warm")
            nc.vector.memzero(warm[:])
            nc.scalar.activation(warm[:], warm[:], f_exp, bias=0.0)

            jw = N // 2
            G2 = 2 * GROUP_PAIRS          # cols per group (2048)
            PW = H * G2                   # cols per chunk tile (8192)
            I_PAIR = H * I_PER_GROUP      # 32 query nodes per chunk tile
            xe4 = xte_t[:, None, None, :].broadcast_to([D, H, I_PER_GROUP, jw])
            xo4 = xto_t[:, None, None, :].broadcast_to([D, H, I_PER_GROUP, jw])

            def stage1_half(rt, e1w, h):
                """mm1 + Exp for one group (half pair)."""
                g0 = h * GROUP_PAIRS
                # f1 layout: [even 0:GROUP | odd GROUP:2*GROUP]
                # s outermost so chunk 0's piecewise DMA unblocks the first
                # two matmuls together (PE runs its queue in order).
                f1 = f1_pool.tile([D, G2], F32, tag="f1")
                for s in range(GROUP_PAIRS // SUB):
                    cs = g0 + s * SUB
                    nc.tensor.matmul(
                        f1[:, s * SUB:(s + 1) * SUB],
                        w1s_t[0:RBF, :],
                        rt[0:RBF, cs:cs + SUB],
                    )
                    nc.tensor.matmul(
                        f1[:, GROUP_PAIRS + s * SUB:
                            GROUP_PAIRS + (s + 1) * SUB],
                        w1s_t[RBF:2 * RBF, :],
                        rt[RBF:2 * RBF, cs:cs + SUB],
                    )
                # softplus(z1 + b1) = Ln(Exp(z1 + b1) + 1)
                nc.scalar.activation(
                    e1w[:, h * G2:(h + 1) * G2], f1[:], f_exp,
                    bias=b1_t[:])

            def stage2_half(a1w, e2w, h, pool=None, ptag="f2"):
                """mm2 + Exp for one group of the previous pair."""
                f2 = (pool or f2_pool).tile([D, G2], F32, tag=ptag)
                for s in range(G2 // SUB):
                    nc.tensor.matmul(
                        f2[:, s * SUB:(s + 1) * SUB],
                        w2_t[:],
                        a1w[:, h * G2 + s * SUB:h * G2 + (s + 1) * SUB],
                    )
                nc.scalar.activation(
                    e2w[:, h * G2:(h + 1) * G2], f2[:], f_exp, bias=b2p_t[:])

            def stage2_tail(e2w, i0):
                """wide Ln of layer 2 + weighted j-reduction (one chunk).

                prod = a2 * xT (x broadcast over h and i), even+odd halves
                pre-added with a TT add (runs 2-4x faster per element than
                TensorReduce, which is stuck at 1 elem/lane/cycle), then one
                half-width segmented reduce over j; finally + corr.
                a2w cols = [h: [even 1024 | odd 1024]] * H."""
                a2w = a2_pool.tile([D, PW], BF16, tag="a2")
                nc.scalar.activation(a2w[:], e2w[:], f_ln, bias=1.0)

                prod = prod_pool.tile([D, PW], BF16, tag="prod", bufs=1)
                p4 = prod[:].rearrange(
                    "p (h par k j) -> p h par k j", h=H, par=2, j=jw)
                a4 = a2w[:].rearrange(
                    "p (h par k j) -> p h par k j", h=H, par=2, j=jw)
                nc.vector.tensor_tensor(
                    p4[:, :, 0, :, :], a4[:, :, 0, :, :], xe4, mult)
                nc.vector.tensor_tensor(
                    p4[:, :, 1, :, :], a4[:, :, 1, :, :], xo4, mult)
                ps = prod_pool.tile([D, PW // 2], BF16, tag="psm")
                nc.vector.tensor_add(
                    ps[:].rearrange("p (h k j) -> p h k j", h=H, j=jw),
                    p4[:, :, 0, :, :], p4[:, :, 1, :, :])
                # one more j-halving on the TT path (0.54 ns/elem) before
                # the reduce (stuck at 1.06 ns/elem regardless of dtype)
                ph = prod_pool.tile([D, PW // 4], BF16, tag="psh")
                p3 = ps[:].rearrange("p (m j) -> p m j", j=jw)
                nc.vector.tensor_add(
                    ph[:].rearrange("p (m j) -> p m j", j=jw // 2),
                    p3[:, :, 0:jw // 2], p3[:, :, jw // 2:jw])
                sums = acc_pool.tile([D, I_PAIR], F32, tag="sums")
                nc.vector.tensor_reduce(
                    sums[:],
                    ph[:].rearrange("p (m j) -> p m j", j=jw // 2),
                    axis=mybir.AxisListType.X,
                    op=add,
                )
                nc.vector.tensor_scalar_add(
                    out_sb[:, i0:i0 + I_PAIR], sums[:], corr_t[:])

            def stage2_tail_fine(e2w, i0, h):
                """Per-half-group (even/odd) Ln + contraction: the post-ACT
                DVE overhang at the kernel tail is one 1024-col piece."""
                prods = []
                for par, xt in ((0, xte_t), (1, xto_t)):
                    cbase = h * G2 + par * GROUP_PAIRS
                    a2h = a2_pool.tile([D, GROUP_PAIRS], BF16, tag="a2f",
                                       bufs=4)
                    nc.scalar.activation(
                        a2h[:], e2w[:, cbase:cbase + GROUP_PAIRS],
                        f_ln, bias=1.0)
                    prod = prod_pool.tile([D, GROUP_PAIRS], BF16, tag="prodf",
                                          bufs=2)
                    xb = xt[:, None, :].broadcast_to([D, I_PER_GROUP, jw])
                    nc.vector.tensor_tensor(
                        prod[:].rearrange("p (k j) -> p k j", j=jw),
                        a2h[:].rearrange("p (k j) -> p k j", j=jw),
                        xb, mult)
                    prods.append(prod)
                psf = prod_pool.tile([D, GROUP_PAIRS], BF16, tag="psf",
                                     bufs=2)
                nc.vector.tensor_add(psf[:], prods[0][:], prods[1][:])
                phf = prod_pool.tile([D, GROUP_PAIRS // 2], BF16, tag="phf",
                                     bufs=2)
                s3 = psf[:].rearrange("p (k j) -> p k j", j=jw)
                nc.vector.tensor_add(
                    phf[:].rearrange("p (k j) -> p k j", j=jw // 2),
                    s3[:, :, 0:jw // 2], s3[:, :, jw // 2:jw])
                sums = acc_pool.tile([D, I_PER_GROUP], F32, tag="sumsf",
                                     bufs=4)
                nc.vector.tensor_reduce(
                    sums[:],
                    phf[:].rearrange("p (k j) -> p k j", j=jw // 2),
                    axis=mybir.AxisListType.X,
                    op=add,
                )
                i0a = i0 + h * I_PER_GROUP
                nc.vector.tensor_scalar_add(
                    out_sb[:, i0a:i0a + I_PER_GROUP], sums[:], corr_t[:])

            # Software-pipelined emission interleaving halves of pair p's
            # layer 1 with halves of pair p-1's layer 2, so every ACT op has
            # a PE window in front of it and ACT never head-of-line blocks.
            NCHUNK = PAIRS // CHUNK_PAIRS

            def body():
                pending = None  # (a1w, i0) of the previous pair
                for c in range(NCHUNK):
                    rt = rt_pool.tile([2 * RBF, CHUNK_PAIRS], BF16, tag="rt")
                    c0 = c * CHUNK_PAIRS
                    if c == 0:
                        # Piecewise first load so mm1 of sub 0 starts as
                        # soon as the first 128 KiB lands; const blobs
                        # interleave so every startup dep lands early.
                        # HWDGE serializes all queues, so order by need:
                        # cb16 (gates mm1 weights) then the first group's
                        # pairs, then cf32 (first Exp bias), then the rest.
                        nc.sync.dma_start(cb16_t[:], cb16[:])
                        nc.sync.dma_start(rt[:, 0:1024], rpt[:, 0:1024])
                        nc.sync.dma_start(cf32_t[:], cf32[:])
                        for (a, b) in ((1024, 2048), (2048, CHUNK_PAIRS)):
                            nc.sync.dma_start(rt[:, a:b], rpt[:, a:b])
                    else:
                        nc.sync.dma_start(
                            rt[:], rpt[:, c0:c0 + CHUNK_PAIRS])
                    e1w = e1_pool.tile([D, PW], F32, tag="e1")
                    if pending is not None:
                        e2w = e2_pool.tile([D, PW], F32, tag="e2")
                    else:
                        e2w = None
                    a1w = a1_pool.tile([D, PW], BF16, tag="a1")
                    for h in range(H):
                        stage1_half(rt, e1w, h)
                        if pending is not None:
                            stage2_half(pending[0], e2w, h)
                        elif h == 0:
                            # chunk 0 has no stage-2 work to overlap; issue
                            # the g0 Ln piece early so iteration 1's mm2 can
                            # start during Exp1(g1..g3) instead of stalling
                            # the first Exp2 behind a cold-PE mm2 chain.
                            nc.scalar.activation(
                                a1w[:, 0:G2], e1w[:, 0:G2], f_ln, bias=1.0)
                    # Ln2 of the previous chunk goes FIRST so its DVE
                    # contraction blob starts ~7us earlier and clears the
                    # vector engine before the flush needs it.
                    if pending is not None:
                        if c == NCHUNK - 1:
                            # spread the last steady chunk's DVE work so the
                            # flush below starts with an empty DVE queue
                            for h in range(H):
                                stage2_tail_fine(e2w, pending[1], h)
                        else:
                            stage2_tail(e2w, pending[1])
                    # Ln1 split [g0 | g1:]: mm2 of the NEXT iteration's
                    # stage-2 only needs its group's a1w slice, so a small
                    # first piece hands PE work ~12 us earlier and keeps it
                    # out of the cold p-state at the chunk boundary.
                    if pending is not None:
                        nc.scalar.activation(
                            a1w[:, 0:G2], e1w[:, 0:G2], f_ln, bias=1.0)
                    nc.scalar.activation(
                        a1w[:, G2:], e1w[:, G2:], f_ln, bias=1.0)
                    pending = (a1w, c * I_PAIR)
                # flush last chunk tile: no stage1 work remains to overlap,
                # so double-buffer the mm2->Exp2 chain across BOTH psum pools
                # (f1's banks are free once its last Exp is done), and run
                # the Ln + j-contraction per group so the post-ACT DVE tail
                # is one group (~3 us) instead of the whole chunk (~13 us).
                e2w = e2_pool.tile([D, PW], F32, tag="e2")
                for h in range(H):
                    if h % 2 == 0:
                        stage2_half(pending[0], e2w, h)
                    else:
                        stage2_half(pending[0], e2w, h,
                                    pool=f1_pool, ptag="f1")
                    stage2_tail_fine(e2w, pending[1], h)

            if reps == 1:
                body()
            else:
                with tc.For_i(0, reps, 1):
                    body()

            # Split the store: [0:224] covers chunks 0-6 whose tails finish
            # well before the flush, so only the last 32 cols gate the end.
            nc.sync.dma_start(outT[:, 0:224], out_sb[:, 0:224])
            nc.sync.dma_start(outT[:, 224:N], out_sb[:, 224:N])

    nc.compile()
    return nc


def _prepare_inputs(x, r, W1, b1, W2, b2):
    bf16 = ml_dtypes.bfloat16
    W1 = np.asarray(W1, np.float32)
    W2 = np.asarray(W2, np.float32)
    w1s = np.concatenate([W1, W1], axis=0).astype(bf16)          # [128, 128]
    w2b = W2.astype(bf16)                                        # [128, 128]
    b1c = np.asarray(b1, np.float32).reshape(D, 1)
    b2p = (np.asarray(b2, np.float32)
           - LOG2 * W2.sum(axis=0)).reshape(D, 1)

    in_maps = []
    for b in range(B):
        xbT = np.asarray(x[b], np.float32).T                     # [128 d, 256 j]
        rpb = np.asarray(r[b], np.float32).reshape(
            PAIRS, 2 * RBF).astype(bf16)
        corr = (-LOG2 * xbT.sum(axis=1, dtype=np.float64)
                ).astype(np.float32).reshape(D, 1)
        cb16 = np.concatenate([
            w1s,
            w2b,
            np.ascontiguousarray(xbT[:, 0::2]).astype(bf16),
            np.ascontiguousarray(xbT[:, 1::2]).astype(bf16),
        ], axis=1)                                               # [128, 512]
        cf32 = np.concatenate([b1c, b2p, corr], axis=1)          # [128, 3]
        in_maps.append({
            "rpt": np.ascontiguousarray(rpb.T),                  # [128, 32768]
            "cb16": cb16,
            "cf32": cf32,
        })
    return in_maps


_NC_CACHE = None


def _get_nc():
    global _NC_CACHE
    if _NC_CACHE is None:
        _NC_CACHE = _build_program()
    return _NC_CACHE


def hw_time_ns(inputs, reps=33, n_meas=3):
    """Measure on-device per-iteration time by comparing wall time of a
    reps-times device loop against a single-iteration run."""
    import time as _time
    in_maps = _prepare_inputs(**inputs)

    def run_with(nc_prog):
        ts = []
        for _ in range(n_meas):
            t0 = _time.time()
            run_bass_kernel_spmd(nc_prog, in_maps, list(range(N_CORES)))
            ts.append(_time.time() - t0)
        return min(ts)

    nc1 = _build_program(reps=1)
    ncr = _build_program(reps=reps)
    w1 = run_with(nc1)
    wr = run_with(ncr)
    return (wr - w1) / (reps - 1) * 1e9


def kernel(x, r, W1, b1, W2, b2, _trace=False, _trace_kwargs=None):
    nc = _get_nc()
    in_maps = _prepare_inputs(x, r, W1, b1, W2, b2)
    res = run_bass_kernel_spmd(
        nc, in_maps, list(range(N_CORES)),
        trace=_trace, **(_trace_kwargs or {}),
    )
    out = np.stack([
        np.asarray(res.results[b]["outT"], np.float32).T for b in range(B)
    ])
    if _trace:
        return out, res
    return out

